# revision 1
# baseline (speedup 1.0000x reference)
"""SchNet forward on 8 Trainium2 NeuronCores (Bass/Tile), data-parallel over molecules.

kernel(**inputs) takes FULL inputs (as produced by setup_inputs) and returns the
FULL [256] float32 per-molecule energies. Shards 256 molecules into 8 groups of
32 (1024 atoms each), runs an SPMD Bass kernel on cores 0-7, gathers outputs.

Edge-filter compression: the per-edge filter W(d)*ccut(d) (filter-MLP on
gaussian-smeared distances times cosine cutoff) is a smooth function of the
single scalar d, so it is fitted host-side per layer onto a 32-gaussian basis
B_b(d) = exp(-beta (d - mu_b)^2):  Phi_l(d) ~= sum_b C_l[b,:] B_b(d).
On device the basis is evaluated once (phase A) and each layer's edge stage is
one [32 -> 100] GEMM + broadcast-multiply by x1 + 32-group sum. Non-selected /
masked edge slots get d = 16, where every basis gaussian underflows to exactly
0.0, reproducing the reference's exact zero weight (ccut(CUTOFF) = 0).

Edge layout: edge slot e = 32*i + j (i = target atom, j = in-molecule neighbor
slot). The basis lives 2-stacked [64, E/2]: partitions 32*(c%2)+b, column block
8192*(c//2) hold basis b of edge chunk c (8192 edges each).
"""

import math
import numpy as np

N = 8192
APM = 32
FEAT = 100
NG = 25
K = 28
L = 4
CUTOFF = 6.0
NCORES = 8
NA = N // NCORES          # atoms per core = 1024
NM = NA // APM            # molecules per core = 32
E = NA * APM              # edge slots per core = 32768
NBLK = NA // 128          # 8 atom blocks per core
H = FEAT // 2

NB = 24                   # gaussian basis size (stacked in 32-row slots)
MU_LO, MU_HI = -0.45, 6.45
MUS = np.linspace(MU_LO, MU_HI, NB)
BETA = 1.0 / (MUS[1] - MUS[0]) ** 2
DPAD = 16.0               # padded-edge distance: all basis gaussians underflow to 0
LOG2 = float(np.log(2.0))
EDGE_DT = "f32"           # dtype of basis/cmat for the edge GEMM

_COMPILED = None


def _build(repeats: int = 1, skip=()):
    import concourse.bass as bass
    import concourse.mybir as mybir
    import concourse.tile as tile
    from concourse import bacc

    skip = set(skip)
    dt = mybir.dt
    F32 = dt.float32
    EDT = {"f32": dt.float32, "bf16": dt.bfloat16, "fp16": dt.float16}[EDGE_DT]
    A = mybir.ActivationFunctionType
    OP = mybir.AluOpType
    AX = mybir.AxisListType
    LF = L * FEAT

    nc = bacc.Bacc()

    pos_d = nc.dram_tensor("pos", [NA, 3], F32, kind="ExternalInput")
    h0_d = nc.dram_tensor("h0", [FEAT, NA], F32, kind="ExternalInput")
    cmat_d = nc.dram_tensor("cmat", [64, LF], EDT, kind="ExternalInput")
    l1w_d = nc.dram_tensor("l1w", [FEAT, LF], F32, kind="ExternalInput")
    l2w_d = nc.dram_tensor("l2w", [FEAT, LF], F32, kind="ExternalInput")
    lww_d = nc.dram_tensor("lww", [FEAT, LF], F32, kind="ExternalInput")
    l2b_d = nc.dram_tensor("l2b", [FEAT, L], F32, kind="ExternalInput")
    lbb_d = nc.dram_tensor("lbb", [FEAT, L], F32, kind="ExternalInput")
    ow1_d = nc.dram_tensor("ow1", [FEAT, H], F32, kind="ExternalInput")
    ob1_d = nc.dram_tensor("ob1", [H, 1], F32, kind="ExternalInput")
    ow2_d = nc.dram_tensor("ow2", [H, 1], F32, kind="ExternalInput")
    ob2_d = nc.dram_tensor("ob2", [1, 1], F32, kind="ExternalInput")
    mus_d = nc.dram_tensor("mus128", [128, 1], F32, kind="ExternalInput")
    diag_d = nc.dram_tensor("diagm", [128, NBLK * APM], F32, kind="ExternalInput")

    out_d = nc.dram_tensor("energy", [NM], F32, kind="ExternalOutput")
    dtl_d = nc.dram_tensor("dtl_lin", [E], F32)

    def bap(a, off, dims):
        return bass.AP(tensor=a.tensor, offset=a.offset + off, ap=dims)

    with tile.TileContext(nc) as tc:
        import contextlib
        ctx = contextlib.ExitStack()
        with ctx:
            persist = ctx.enter_context(tc.tile_pool(name="persist", bufs=1))
            ps = ctx.enter_context(tc.tile_pool(name="ps", bufs=1, space="PSUM"))
            sa = ctx.enter_context(tc.tile_pool(name="scrA", bufs=1))
            sb = ctx.enter_context(tc.tile_pool(name="scrB", bufs=1))

            basis = persist.tile([64, E // 2], EDT, tag="basis")
            hA = persist.tile([FEAT, NA], F32, tag="hA")
            hB = persist.tile([FEAT, NA], F32, tag="hB")
            x1_t = persist.tile([FEAT, NA], dt.bfloat16, tag="x1")
            agg_t = persist.tile([FEAT, NA], F32, tag="agg")

            cmat_t = persist.tile([64, LF], EDT, tag="cmat")
            l1w_t = persist.tile([FEAT, LF], F32, tag="l1w")
            l2w_t = persist.tile([FEAT, LF], F32, tag="l2w")
            lw_t = persist.tile([FEAT, LF], F32, tag="lww")
            l2b_t = persist.tile([FEAT, L], F32, tag="l2b")
            lb_t = persist.tile([FEAT, L], F32, tag="lb")
            ow1_t = persist.tile([FEAT, H], F32, tag="ow1")
            ob1_t = persist.tile([H, 1], F32, tag="ob1")
            ow2_t = persist.tile([H, 1], F32, tag="ow2")
            ob2_t = persist.tile([1, 1], F32, tag="ob2")
            mus_t = persist.tile([128, 1], F32, tag="mus")
            diag_t = persist.tile([128, NBLK * APM], F32, tag="diag")
            half_t = persist.tile([128, 1], F32, tag="half")
            cstage = persist.tile([64, FEAT], EDT, tag="cstage")
            l1s = persist.tile([FEAT, FEAT], F32, tag="l1s")
            l2s = persist.tile([FEAT, FEAT], F32, tag="l2s")
            lws = persist.tile([FEAT, FEAT], F32, tag="lws")
            nc.vector.memset(half_t[:], 0.5)
            if "phA" in skip or "basis" in skip:
                nc.vector.memset(basis[:], 0.5)
            if "multred" in skip:
                nc.vector.memset(agg_t[:], 1.0)
            if "x1g" in skip:
                nc.vector.memset(x1_t[:], 1.0)

            nc.sync.dma_start(out=cmat_t[:], in_=cmat_d[:])
            nc.sync.dma_start(out=l1w_t[:], in_=l1w_d[:])
            nc.sync.dma_start(out=l2w_t[:], in_=l2w_d[:])
            nc.sync.dma_start(out=lw_t[:], in_=lww_d[:])
            nc.sync.dma_start(out=l2b_t[:], in_=l2b_d[:])
            nc.sync.dma_start(out=lb_t[:], in_=lbb_d[:])
            nc.sync.dma_start(out=ow1_t[:], in_=ow1_d[:])
            nc.sync.dma_start(out=ob1_t[:], in_=ob1_d[:])
            nc.sync.dma_start(out=ow2_t[:], in_=ow2_d[:])
            nc.sync.dma_start(out=ob2_t[:], in_=ob2_d[:])
            nc.sync.dma_start(out=mus_t[:], in_=mus_d[:])
            nc.sync.dma_start(out=diag_t[:], in_=diag_d[:])

            for rep in range(repeats):
                nc.sync.dma_start(out=hA[:], in_=h0_d[:])
                if "phA" not in skip:
                    # ------- phase A: contiguous mapping, partition p = atoms 8p..8p+7
                    posA = sa.tile([128, NBLK, 3], F32, tag="posA")
                    nc.sync.dma_start(
                        out=posA[:], in_=bap(pos_d[:], 0, [[24, 128], [1, 24]]))
                    posB = sa.tile([128, APM, 3], F32, tag="posB")
                    nc.sync.dma_start(
                        out=posB[:], in_=bap(pos_d[:], 0, [[96, 32], [0, 4], [1, 96]]))
                    dif = sa.tile([128, NBLK, APM, 3], F32, tag="dif")
                    pB, pA = posB[:], posA[:]
                    nc.vector.tensor_tensor(
                        out=dif[:],
                        in0=bap(pB, 0, [pB.ap[0], [0, NBLK], [3, APM], [1, 3]]),
                        in1=bap(pA, 0, [pA.ap[0], [3, NBLK], [0, APM], [1, 3]]),
                        op=OP.subtract)
                    sq = sa.tile([128, NBLK, APM, 3], F32, tag="sq")
                    nc.vector.tensor_tensor(out=sq[:], in0=dif[:], in1=dif[:],
                                            op=OP.mult)
                    d2 = sa.tile([128, NBLK * APM], F32, tag="d2")
                    nc.vector.tensor_reduce(
                        out=d2[:], in_=sq[:].rearrange("p b a c -> p (b a) c"),
                        axis=AX.X, op=OP.add)
                    # self-edges to huge, then clamp everything masked to 36
                    nc.vector.tensor_tensor(out=d2[:], in0=d2[:], in1=diag_t[:],
                                            op=OP.add)
                    d2c = sa.tile([128, NBLK * APM], F32, tag="d2c")
                    nc.vector.tensor_scalar(out=d2c[:], in0=d2[:], scalar1=36.0,
                                            scalar2=None, op0=OP.min)
                    # rank by counting strictly-smaller entries within each row
                    dd = d2c[:]
                    rank = sa.tile([128, NBLK * APM], F32, tag="rank")
                    NH = NBLK // 2
                    for h2 in range(2):
                        lt = sa.tile([128, NH * APM * APM], F32, tag="lt")
                        o2 = NH * APM * h2
                        nc.vector.tensor_tensor(
                            out=lt[:],
                            in0=bap(dd, o2, [dd.ap[0], [APM, NH], [0, APM], [1, APM]]),
                            in1=bap(dd, o2, [dd.ap[0], [APM, NH], [1, APM], [0, APM]]),
                            op=OP.is_lt)
                        nc.vector.tensor_reduce(
                            out=rank[:, o2:o2 + NH * APM],
                            in_=lt[:].rearrange("p (a j) -> p a j", j=APM),
                            axis=AX.X, op=OP.add)
                    sel = sa.tile([128, NBLK * APM], F32, tag="sel")
                    nc.vector.tensor_scalar(out=sel[:], in0=rank[:],
                                            scalar1=float(K) - 0.5, scalar2=None,
                                            op0=OP.is_lt)
                    s_t = sa.tile([128, NBLK * APM], F32, tag="s_t")
                    nc.scalar.activation(s_t[:], d2c[:], A.Sqrt)
                    # dtil = sel ? d : DPAD  ==  (d - DPAD)*sel + DPAD
                    dt1 = sa.tile([128, NBLK * APM], F32, tag="dt1")
                    nc.vector.scalar_tensor_tensor(
                        out=dt1[:], in0=s_t[:], scalar=-DPAD, in1=sel[:],
                        op0=OP.add, op1=OP.mult)
                    dtil = sa.tile([128, NBLK * APM], F32, tag="dtil")
                    nc.vector.tensor_scalar(out=dtil[:], in0=dt1[:], scalar1=DPAD,
                                            scalar2=None, op0=OP.add)
                    # linear edge order e = 256p + 32a + j: contiguous per partition
                    nc.sync.dma_start(
                        out=bap(dtl_d[:], 0, [[256, 128], [1, 256]]),
                        in_=dtil[:])
                    # 2-stacked broadcast, built in 4 chunks of [64, 4096]
                    CH = 4096
                    for chk in range(4):
                        off = (E // 2) * (chk // 2) + CH * (chk % 2)
                        drep = sa.tile([64, CH], F32, tag="drep")
                        nc.sync.dma_start(
                            out=drep[:],
                            in_=bap(dtl_d[:], off, [[E // 4, 2], [0, 32], [1, CH]]))
                        q = sa.tile([64, CH], F32, tag="q")
                        nc.vector.tensor_scalar(out=q[:], in0=drep[:],
                                                scalar1=mus_t[:64], scalar2=None,
                                                op0=OP.subtract)
                        q2 = sa.tile([64, CH], F32, tag="q2")
                        nc.vector.tensor_tensor(out=q2[:], in0=q[:], in1=q[:],
                                                op=OP.mult)
                        nc.scalar.activation(
                            basis[:, CH * chk:CH * (chk + 1)],
                            q2[:], A.Exp, scale=-float(BETA))

                # ---------- phase B: interaction layers ----------
                hcur, hnxt = hA, hB
                for l in range(L):
                    lf = slice(FEAT * l, FEAT * (l + 1))
                    nc.vector.tensor_copy(cstage[:], cmat_t[:, lf])
                    nc.vector.tensor_copy(l1s[:], l1w_t[:, lf])
                    nc.vector.tensor_copy(l2s[:], l2w_t[:, lf])
                    nc.vector.tensor_copy(lws[:], lw_t[:, lf])
                    if "x1g" not in skip:
                        psx = ps.tile([FEAT, 4096], F32, tag="ps")
                        for hh in range(2):
                            qs = slice(512 * hh, 512 * (hh + 1))
                            nc.tensor.matmul(psx[:, qs], l1s[:], hcur[:, qs],
                                             start=True, stop=True)
                        nc.vector.tensor_copy(x1_t[:], psx[:, :NA])

                    for k in range(E // 4096):
                        c, hf = k // 2, k % 2
                        p32 = slice(32 * (c % 2), 32 * (c % 2) + NB)
                        base = (E // 4) * (c // 2) + 4096 * hf
                        pse = ps.tile([FEAT, 4096], F32, tag="ps")
                        for q8 in (range(8) if "edgemm" not in skip else [0]):
                            cs = slice(base + 512 * q8, base + 512 * (q8 + 1))
                            nc.tensor.matmul(pse[:, 512 * q8:512 * (q8 + 1)],
                                             cstage[p32, :], basis[p32, cs],
                                             start=True, stop=True)
                        if "multred" not in skip:
                            msg = sb.tile([FEAT, 4096], dt.bfloat16, tag="msg")
                            xx = x1_t[:]
                            nc.vector.tensor_tensor(
                                out=msg[:], in0=pse[:],
                                in1=bap(xx, 128 * k,
                                        [xx.ap[0], [APM, 4], [0, APM], [1, APM]]),
                                op=OP.mult)
                            nc.vector.tensor_reduce(
                                out=agg_t[:, 128 * k:128 * (k + 1)],
                                in_=msg[:].rearrange("p (a j) -> p a j", j=APM),
                                axis=AX.X, op=OP.add)

                    if "node" not in skip:
                        psv = ps.tile([FEAT, 4096], F32, tag="ps")
                        for hh in range(2):
                            qs = slice(512 * hh, 512 * (hh + 1))
                            nc.tensor.matmul(psv[:, qs], l2s[:], agg_t[:, qs],
                                             start=True, stop=True)
                        spe = sb.tile([FEAT, NA], F32, tag="spe")
                        nc.scalar.activation(spe[:], psv[:, :NA], A.Exp,
                                             bias=l2b_t[:, l:l + 1])
                        spl = sb.tile([FEAT, NA], F32, tag="spl")
                        nc.scalar.activation(spl[:], spe[:], A.Ln,
                                             bias=half_t[:FEAT], scale=0.5)
                        psw = ps.tile([FEAT, 4096], F32, tag="ps")
                        for hh in range(2):
                            qs = slice(512 * hh, 512 * (hh + 1))
                            nc.tensor.matmul(psw[:, qs], lws[:], spl[:, qs],
                                             start=True, stop=True)
                        nc.vector.scalar_tensor_tensor(
                            out=hnxt[:], in0=psw[:, :NA], scalar=lb_t[:, l:l + 1],
                            in1=hcur[:], op0=OP.add, op1=OP.add)
                        hcur, hnxt = hnxt, hcur

                # ---------- phase C: readout ----------
                psr = ps.tile([FEAT, 4096], F32, tag="ps")
                for hh in range(2):
                    qs = slice(512 * hh, 512 * (hh + 1))
                    nc.tensor.matmul(psr[:H, qs], ow1_t[:], hcur[:, qs],
                                     start=True, stop=True)
                re = sb.tile([H, NA], F32, tag="re")
                nc.scalar.activation(re[:], psr[:H, :NA], A.Exp, bias=ob1_t[:])
                rl = sb.tile([H, NA], F32, tag="rl")
                nc.scalar.activation(rl[:], re[:], A.Ln, bias=half_t[:H],
                                     scale=0.5)
                pso = ps.tile([FEAT, 4096], F32, tag="ps")
                for hh in range(2):
                    qs = slice(512 * hh, 512 * (hh + 1))
                    nc.tensor.matmul(pso[:1, qs], ow2_t[:], rl[:, qs],
                                     start=True, stop=True)
                pa = sb.tile([1, NA], F32, tag="pa")
                nc.vector.tensor_scalar(out=pa[:], in0=pso[:1, :NA],
                                        scalar1=ob2_t[:1, :], scalar2=None,
                                        op0=OP.add)
                en = sb.tile([1, NM], F32, tag="en")
                nc.vector.tensor_reduce(
                    out=en[:], in_=pa[:].rearrange("p (m i) -> p m i", i=APM),
                    axis=AX.X, op=OP.add)
                nc.sync.dma_start(out=out_d[:].unsqueeze(0), in_=en[:])

    nc.compile()
    return nc


def _ssp(x):
    return np.logaddexp(0.0, x) - LOG2


def _fit_filters(mlp_w1, mlp_b1, mlp_w2, mlp_b2, ngrid=8000, ridge=1e-8):
    """Fit per-layer C [NB, FEAT] s.t. basis(d) @ C ~= filter(d)*ccut(d) on (0, 6].

    Design matrix reproduces the on-device arithmetic: f32 subtract/square/exp,
    then storage rounding for the edge-GEMM dtype.
    """
    dd = np.linspace(1e-4, CUTOFF, ngrid)
    q = (dd[:, None].astype(np.float32) - MUS[None, :].astype(np.float32))
    q2 = (q * q).astype(np.float32)
    Abf = np.exp((-BETA * q2).astype(np.float32)).astype(np.float32)
    if EDGE_DT == "bf16":
        import ml_dtypes
        Abf = Abf.astype(ml_dtypes.bfloat16)
    elif EDGE_DT == "fp16":
        Abf = Abf.astype(np.float16)
    Abf = Abf.astype(np.float64)

    offset = np.linspace(0.0, CUTOFF, NG)
    coeff = -0.5 / (offset[1] - offset[0]) ** 2
    ea = np.exp(coeff * (dd[:, None] - offset[None, :]) ** 2)
    ccut = 0.5 * (np.cos(dd * np.pi / CUTOFF) + 1.0)

    G = Abf.T @ Abf + ridge * np.eye(NB)
    Cs = []
    for l in range(L):
        T = (_ssp(ea @ mlp_w1[l] + mlp_b1[l]) @ mlp_w2[l] + mlp_b2[l]) * ccut[:, None]
        C = np.linalg.solve(G, Abf.T @ T)
        Cs.append(C)
    return Cs


def _prep_inputs(z, pos, ptr, emb, mlp_w1, mlp_b1, mlp_w2, mlp_b2,
                 lin1_w, lin2_w, lin2_b, lin_w, lin_b,
                 out_w1, out_b1, out_w2, out_b2):
    z = np.asarray(z)
    pos = np.ascontiguousarray(np.asarray(pos, dtype=np.float32))
    ptr = np.asarray(ptr)
    assert pos.shape == (N, 3)
    expect = np.arange(0, N + APM, APM)
    assert np.array_equal(ptr.astype(np.int64), expect), "non-uniform molecules unsupported"

    emb = np.asarray(emb, dtype=np.float32)
    Cs = _fit_filters(np.asarray(mlp_w1, np.float64), np.asarray(mlp_b1, np.float64),
                      np.asarray(mlp_w2, np.float64), np.asarray(mlp_b2, np.float64))
    if EDGE_DT == "bf16":
        import ml_dtypes
        cdt = ml_dtypes.bfloat16
    elif EDGE_DT == "fp16":
        cdt = np.float16
    else:
        cdt = np.float32
    cmat = np.zeros((64, L * FEAT), dtype=cdt)
    for c in range(2):
        for l in range(L):
            cmat[32 * c:32 * c + NB, FEAT * l:FEAT * (l + 1)] = Cs[l].astype(cdt)

    def lstack(w):  # [L, F, F] -> [F, L*F] (contract dim on partitions)
        w = np.asarray(w, np.float32)
        return np.ascontiguousarray(w.transpose(1, 0, 2).reshape(FEAT, L * FEAT))

    mus128 = np.full((128, 1), 10.0, dtype=np.float32)
    for p in range(128):
        if p % 32 < NB:
            mus128[p, 0] = MUS[p % 32]
    diagm = np.zeros((128, NBLK * APM), dtype=np.float32)
    for p in range(128):
        for a in range(NBLK):
            diagm[p, APM * a + 8 * (p % 4) + a] = 1e9

    shared = {
        "cmat": cmat,
        "l1w": lstack(lin1_w),
        "l2w": lstack(lin2_w),
        "lww": lstack(lin_w),
        "l2b": np.ascontiguousarray(np.asarray(lin2_b, np.float32).T),
        "lbb": np.ascontiguousarray(np.asarray(lin_b, np.float32).T),
        "ow1": np.ascontiguousarray(np.asarray(out_w1, np.float32)),
        "ob1": np.asarray(out_b1, np.float32).reshape(H, 1),
        "ow2": np.ascontiguousarray(np.asarray(out_w2, np.float32)),
        "ob2": np.asarray(out_b2, np.float32).reshape(1, 1),
        "mus128": mus128,
        "diagm": diagm,
    }
    in_maps = []
    for c in range(NCORES):
        sl = slice(NA * c, NA * (c + 1))
        h0 = emb[np.asarray(z[sl], dtype=np.int64)].T
        m = dict(shared)
        m["pos"] = pos[sl].copy()
        m["h0"] = np.ascontiguousarray(h0, dtype=np.float32)
        in_maps.append(m)
    return in_maps


def kernel(**inputs) -> np.ndarray:
    from concourse.bass_utils import run_bass_kernel_spmd
    global _COMPILED
    if _COMPILED is None:
        _COMPILED = _build(1)
    nc = _COMPILED
    in_maps = _prep_inputs(**inputs)
    res = run_bass_kernel_spmd(nc, in_maps, list(range(NCORES)))
    out = np.concatenate([res.results[c]["energy"] for c in range(NCORES)])
    return out.astype(np.float32)


if __name__ == "__main__":
    _build(1)
    print("built ok")

